# revision 11
# baseline (speedup 1.0000x reference)
"""Bahdanau-attention score kernel (softmax(v . tanh(W[h;enc]+b))) for 8 TRN2 cores.

Self-contained: hardcodes shapes B=32, S=2048, ENC2=600, DD=900.
Sharding: data-parallel over batch (4 batches/core), weights replicated.

Design (v3):
- Host prep: enc cast to fp16, padded to 640 cols; cols 600..603 carry a
  one-hot(batch) so the h-projection (computed on host, appended to We^T
  rows) lands via 4 extra contraction rows.
- enc chunks land TRANSPOSED in SBUF via the DMA xbar transpose (16-bit
  path, <=496 source rows per transfer to dodge the 512-row ucode bug).
  Weight/v DMAs ride SWDGE on the idle GPSIMD queue; xbar issues are
  split between SP (need-ordered) and ACT (woven into the loop) because
  a DMA_TRANSPOSE issue slice costs ~1.3us on its queue.
- PE runs only the 10 contraction matmuls per 128-row tile; ACT drains
  PSUM with tanh (fp16 out); DVE does fp16 mul (2x mode) + reduce.
- Softmax is per batch, overlapped: transpose+exp a batch after its las
  tile, normalize via PE ones-matmul sum + reciprocal + PE broadcast
  (no DRAM roundtrip), per-batch output DMA. Only batch 3's tail is
  exposed (~3us).
"""

import os

import numpy as np

import concourse.bass as bass  # noqa: F401
import concourse.mybir as mybir
import concourse.tile as tile
from concourse import bacc
from concourse.bass_utils import run_bass_kernel_spmd
from concourse.masks import make_identity

F32 = mybir.dt.float32
F16 = mybir.dt.float16
AF = mybir.ActivationFunctionType
ALU = mybir.AluOpType
AX = mybir.AxisListType

NCORES = 8
B, S, E2, DD = 32, 2048, 600, 900
EP = 640                    # padded e dim (5 xbar chunks of 128)
BL = B // NCORES            # 4 batches per core
SROWS = BL * S              # 8192 s-rows per core
P = 128
NTIL = SROWS // P           # 64 score tiles/columns
TPB = S // P                # 16 tiles per batch
NCH = 5                     # e chunks of 128 (last: 88 enc + 4 one-hot)
K4 = 92                     # chunk-4 contraction rows
# xbar transpose segments: the ucode instruction caps at 512 source rows
# and corrupts the tail at exactly 512, so use 496-row segments
SEGS = [(0, 496), (496, 496), (992, 496), (1488, 496), (1984, 64)]

K_TILES = int(os.environ.get("K_TILES", NTIL))
K_WEAVE = int(os.environ.get("K_WEAVE", "1"))

# segs issued from ACT between tiles of the previous batch: (c, g) pairs
ACT_WEAVE = ([(0, 0), (1, 0), (2, 0), (3, 0), (4, 0), (0, 1), (1, 1), (2, 1)]
             if K_WEAVE else [])


def build():
    nc = bacc.Bacc("TRN2", target_bir_lowering=False)
    enc_ext = nc.dram_tensor("enc", [SROWS, EP], F16, kind="ExternalInput")
    wcat_ext = nc.dram_tensor("wcat", [512 + K4, DD], F16, kind="ExternalInput")
    v_ext = nc.dram_tensor("v", [1, DD], F16, kind="ExternalInput")
    out_ext = nc.dram_tensor("out", [BL, S], F32, kind="ExternalOutput")

    with tile.TileContext(nc) as tc:
        with (
            tc.tile_pool(name="stat", bufs=1) as stat,
            tc.tile_pool(name="encp", bufs=BL) as encp,
            tc.tile_pool(name="zp", bufs=3) as zp,
            tc.tile_pool(name="jp", bufs=2) as jp,
            tc.tile_pool(name="ps_e", bufs=3, space="PSUM") as ps_e,
            tc.tile_pool(name="ps_t", bufs=2, space="PSUM") as ps_t,
        ):
            # ---------------- weights via SWDGE on the idle POOL queue ----
            rhs_main = stat.tile([P, 4, DD], F16)
            rhs4 = stat.tile([K4, DD], F16)
            v_rep = stat.tile([P, DD], F16)
            for h in range(4):
                for (no, nn) in ((0, 450), (450, 450)):
                    nc.gpsimd.dma_start(
                        out=rhs_main[:, h, no:no + nn],
                        in_=wcat_ext.ap()[h * P:(h + 1) * P, no:no + nn],
                    )
            nc.gpsimd.dma_start(out=rhs4[:, :], in_=wcat_ext.ap()[512:512 + K4, :])
            for (po, pn) in ((0, 64), (64, 64)):
                nc.gpsimd.dma_start(
                    out=v_rep[po:po + pn, :],
                    in_=v_ext.ap().partition_broadcast(pn),
                )

            # ---------------- enc tiles + xbar transpose issue plan -------
            enc_tiles = {}
            for b in range(BL):
                for c in range(NCH):
                    enc_tiles[(b, c)] = encp.tile(
                        [P, S], F16, tag=f"enc{c}", name=f"enc{b}_{c}"
                    )

            def emit_seg(eng, b, c, g):
                g0, gn = SEGS[g]
                eng.dma_start(
                    out=enc_tiles[(b, c)][:, g0:g0 + gn],
                    in_=enc_ext.ap()[b * S + g0:b * S + g0 + gn,
                                     c * P:(c + 1) * P],
                    transpose=True,
                )

            act_set = {(b + 1, c, g) for b in range(BL - 1) for (c, g) in ACT_WEAVE}
            sp_list = sorted(
                ((b, c, g)
                 for b in range(BL) for c in range(NCH) for g in range(len(SEGS))
                 if (b, c, g) not in act_set),
                key=lambda t: (30.0 * t[0] + 7.5 * t[2], t[1]),
            )
            for (b, c, g) in sp_list:
                emit_seg(nc.sync, b, c, g)

            # ---------------- constants ----------------
            ident_f = stat.tile([P, P], F32)
            make_identity(nc, ident_f[:, :])
            ones16 = stat.tile([TPB, 1], F32)
            nc.vector.memset(ones16[:, :], 1.0)
            ones1x16 = stat.tile([1, TPB], F32)
            nc.vector.memset(ones1x16[:, :], 1.0)

            scores = stat.tile([P, NTIL], F32)
            e1 = stat.tile([TPB, BL, P], F32)
            rs = stat.tile([TPB, BL], F32)
            rbi = stat.tile([1, BL], F32)
            outf = stat.tile([TPB, BL, P], F32)

            # ---------------- per-batch softmax pieces ----------------
            def emit_exp(b):
                c0 = b * TPB
                pst = ps_t.tile([P, P], F32, tag="tp", name=f"pst{b}")
                nc.tensor.transpose(
                    pst[0:TPB, :], scores[:, c0:c0 + TPB], ident_f[:, :]
                )
                nc.scalar.activation(
                    e1[:, b, :], pst[0:TPB, :], AF.Exp,
                    accum_out=rs[:, b:b + 1],
                )

            def emit_tail(b):
                zb = ps_t.tile([P, P], F32, tag="tp", name=f"zb{b}")
                nc.tensor.matmul(zb[0:1, 0:1], ones16[:, :], rs[:, b:b + 1])
                nc.vector.reciprocal(rbi[:, b:b + 1], zb[0:1, 0:1])
                rfacp = ps_t.tile([P, P], F32, tag="tp", name=f"rf{b}")
                nc.tensor.matmul(rfacp[0:TPB, 0:1], ones1x16[:, :],
                                 rbi[:, b:b + 1])
                nc.vector.tensor_scalar_mul(
                    outf[:, b, :], e1[:, b, :], rfacp[0:TPB, 0:1]
                )
                nc.sync.dma_start(
                    out=out_ext.ap()[b:b + 1, :].rearrange(
                        "b (t p) -> (b t) p", p=P),
                    in_=outf[:, b, :],
                )

            # ---------------- main loop ----------------
            weave = {b: list(ACT_WEAVE) for b in range(BL - 1)}
            for t in range(K_TILES):
                b, ti = divmod(t, TPB)
                eps = ps_e.tile([P, DD], F32, tag="ep")
                for c in range(NCH):
                    et = enc_tiles[(b, c)]
                    kk = P if c < 4 else K4
                    rr = rhs_main[:, c, :] if c < 4 else rhs4[:, :]
                    for (no, nn) in ((0, 512), (512, 388)):
                        nc.tensor.matmul(
                            eps[:, no:no + nn],
                            et[0:kk, ti * P:(ti + 1) * P],
                            rr[:, no:no + nn],
                            start=(c == 0), stop=(c == NCH - 1),
                        )
                z = zp.tile([P, DD], F16, tag="z")
                nc.scalar.activation(z[:, :], eps[:, :], AF.Tanh)
                junk = jp.tile([P, DD], F16, tag="junk")
                nc.vector.tensor_mul(junk[:, :], z[:, :], v_rep[:, :])
                nc.vector.tensor_reduce(
                    out=scores[:, t:t + 1], in_=junk[:, :],
                    axis=AX.X, op=ALU.add,
                )

                if K_TILES != NTIL:
                    continue
                # ACT-woven xbar issues for the next batch
                if b < BL - 1 and ti % 2 == 1 and weave[b]:
                    (c_, g_) = weave[b].pop(0)
                    emit_seg(nc.scalar, b + 1, c_, g_)
                # overlapped softmax for the previous batch
                if b >= 1 and ti == 1:
                    emit_exp(b - 1)
                if b >= 1 and ti == 6:
                    emit_tail(b - 1)

            if K_TILES < NTIL:
                return nc

            emit_exp(BL - 1)
            emit_tail(BL - 1)
    return nc


_CACHE = {}


def _get_nc():
    if "nc" not in _CACHE:
        nc = build()
        nc.compile()
        _CACHE["nc"] = nc
    return _CACHE["nc"]


def make_in_maps(hidden, encoder_outputs, attn_W, attn_b, v):
    hidden = np.asarray(hidden, dtype=np.float32)
    encoder_outputs = np.asarray(encoder_outputs, dtype=np.float32)
    attn_W = np.asarray(attn_W, dtype=np.float32)
    attn_b = np.asarray(attn_b, dtype=np.float32)
    v = np.asarray(v, dtype=np.float32)

    WeT = np.ascontiguousarray(attn_W[:, DD:].T)          # [600, 900]
    hb_all = hidden @ attn_W[:, :DD].T + attn_b           # [32, 900]
    v16 = v.astype(np.float16).reshape(1, DD)

    in_maps = []
    for c in range(NCORES):
        bs = slice(c * BL, (c + 1) * BL)
        encp = np.zeros((SROWS, EP), dtype=np.float16)
        encp[:, :E2] = encoder_outputs[bs].reshape(SROWS, E2)
        for b in range(BL):
            encp[b * S:(b + 1) * S, E2 + b] = 1.0
        wcat = np.concatenate([WeT, hb_all[bs]], axis=0).astype(np.float16)
        in_maps.append({
            "enc": encp,
            "wcat": np.ascontiguousarray(wcat),
            "v": v16,
        })
    return in_maps


def run(in_maps, trace=False, **kw):
    nc = _get_nc()
    return run_bass_kernel_spmd(nc, in_maps, core_ids=list(range(NCORES)),
                                trace=trace, **kw)


def kernel(hidden, encoder_outputs, attn_W, attn_b, v):
    in_maps = make_in_maps(hidden, encoder_outputs, attn_W, attn_b, v)
    try:
        res = run(in_maps)
    except Exception:
        # transient device states (e.g. a previously wedged core) sometimes
        # clear on retry
        res = run(in_maps)
    out = np.concatenate([res.results[c]["out"] for c in range(NCORES)], axis=0)
    return np.ascontiguousarray(out, dtype=np.float32)


# revision 15
# speedup vs baseline: 1.2289x; 1.2289x over previous
"""Bahdanau-attention score kernel (softmax(v . tanh(W[h;enc]+b))) for 8 TRN2 cores.

Self-contained: hardcodes shapes B=32, S=2048, ENC2=600, DD=900.
Sharding: data-parallel over batch (4 batches/core), weights replicated.

Design (v3):
- Host prep: enc cast to fp16, padded to 640 cols; cols 600..603 carry a
  one-hot(batch) so the h-projection (computed on host, appended to We^T
  rows) lands via 4 extra contraction rows.
- enc chunks land TRANSPOSED in SBUF via the DMA xbar transpose (16-bit
  path, <=496 source rows per transfer to dodge the 512-row ucode bug).
  Weight/v DMAs ride SWDGE on the idle GPSIMD queue; xbar issues are
  split between SP (need-ordered) and ACT (woven into the loop) because
  a DMA_TRANSPOSE issue slice costs ~1.3us on its queue.
- PE runs only the 10 contraction matmuls per 128-row tile; ACT drains
  PSUM with tanh (fp16 out); DVE does fp16 mul (2x mode) + reduce.
- Softmax is per batch, overlapped: transpose+exp a batch after its las
  tile, normalize via PE ones-matmul sum + reciprocal + PE broadcast
  (no DRAM roundtrip), per-batch output DMA. Only batch 3's tail is
  exposed (~3us).
"""

import os

import numpy as np

import concourse.bass as bass  # noqa: F401
import concourse.mybir as mybir
import concourse.tile as tile
from concourse import bacc
from concourse.bass_utils import run_bass_kernel_spmd
from concourse.masks import make_identity

F32 = mybir.dt.float32
F16 = mybir.dt.float16
AF = mybir.ActivationFunctionType
ALU = mybir.AluOpType
AX = mybir.AxisListType

NCORES = 8
B, S, E2, DD = 32, 2048, 600, 900
EP = 640                    # padded e dim (5 xbar chunks of 128)
BL = B // NCORES            # 4 batches per core
SROWS = BL * S              # 8192 s-rows per core
P = 128
NTIL = SROWS // P           # 64 score tiles/columns
TPB = S // P                # 16 tiles per batch
NCH = 5                     # e chunks of 128 (last: 88 enc + 4 one-hot)
K4 = 92                     # chunk-4 contraction rows
# xbar transpose slabs: one 3D-output DMA per slab transposes all 5
# e-chunks at once ([rows, 640] -> [128, 5, rows]). The ucode caps at 512
# source rows (and corrupts the tail at exactly 512), and slab boundaries
# must be 128-aligned so matmul lhsT slices stay within one slab tile.
SLABS = [(0, 384), (384, 384), (768, 384), (1152, 384), (1536, 384),
         (1920, 128)]
TPSL = 3                    # s-tiles per 384-row slab

K_TILES = int(os.environ.get("K_TILES", NTIL))


def build():
    nc = bacc.Bacc("TRN2", target_bir_lowering=False)
    enc_ext = nc.dram_tensor("enc", [SROWS, EP], F16, kind="ExternalInput")
    wcat_ext = nc.dram_tensor("wcat", [512 + K4, DD], F16, kind="ExternalInput")
    v_ext = nc.dram_tensor("v", [1, DD], F16, kind="ExternalInput")
    out_ext = nc.dram_tensor("out", [BL, S], F32, kind="ExternalOutput")

    with tile.TileContext(nc) as tc:
        with (
            tc.tile_pool(name="stat", bufs=1) as stat,
            tc.tile_pool(name="encp", bufs=BL) as encp,
            tc.tile_pool(name="zp", bufs=3) as zp,
            tc.tile_pool(name="jp", bufs=2) as jp,
            tc.tile_pool(name="ps_e", bufs=3, space="PSUM") as ps_e,
            tc.tile_pool(name="ps_t", bufs=2, space="PSUM") as ps_t,
        ):
            # ---------------- weights via SWDGE on the idle POOL queue ----
            rhs_main = stat.tile([P, 4, DD], F16)
            rhs4 = stat.tile([K4, DD], F16)
            v_rep = stat.tile([P, DD], F16)
            for h in range(4):
                for (no, nn) in ((0, 450), (450, 450)):
                    nc.gpsimd.dma_start(
                        out=rhs_main[:, h, no:no + nn],
                        in_=wcat_ext.ap()[h * P:(h + 1) * P, no:no + nn],
                    )
            nc.gpsimd.dma_start(out=rhs4[:, :], in_=wcat_ext.ap()[512:512 + K4, :])
            for (po, pn) in ((0, 64), (64, 64)):
                nc.gpsimd.dma_start(
                    out=v_rep[po:po + pn, :],
                    in_=v_ext.ap().partition_broadcast(pn),
                )

            # ------- enc slab tiles + 3D xbar transpose DMAs (all on SP) --
            enc_tiles = {}
            for b in range(BL):
                for g, (g0, gn) in enumerate(SLABS):
                    et = encp.tile([P, NCH, gn], F16, tag=f"slab{g}",
                                   name=f"enc{b}_{g}")
                    nc.sync.dma_start(
                        out=et[:, :, :],
                        in_=enc_ext.ap()[b * S + g0:b * S + g0 + gn, :],
                        transpose=True,
                    )
                    enc_tiles[(b, g)] = et

            # ---------------- constants ----------------
            ident_f = stat.tile([P, P], F32)
            make_identity(nc, ident_f[:, :])
            ones16 = stat.tile([TPB, 1], F32)
            nc.vector.memset(ones16[:, :], 1.0)
            ones1x16 = stat.tile([1, TPB], F32)
            nc.vector.memset(ones1x16[:, :], 1.0)

            scores = stat.tile([P, NTIL], F32)
            e1 = stat.tile([TPB, BL, P], F32)
            rs = stat.tile([TPB, BL], F32)
            rbi = stat.tile([1, BL], F32)
            outf = stat.tile([TPB, BL, P], F32)

            # ---------------- per-batch softmax pieces ----------------
            def emit_exp(b):
                c0 = b * TPB
                pst = ps_t.tile([P, P], F32, tag="tp", name=f"pst{b}")
                nc.tensor.transpose(
                    pst[0:TPB, :], scores[:, c0:c0 + TPB], ident_f[:, :]
                )
                nc.scalar.activation(
                    e1[:, b, :], pst[0:TPB, :], AF.Exp,
                    accum_out=rs[:, b:b + 1],
                )

            def emit_tail(b):
                zb = ps_t.tile([P, P], F32, tag="tp", name=f"zb{b}")
                nc.tensor.matmul(zb[0:1, 0:1], ones16[:, :], rs[:, b:b + 1])
                nc.vector.reciprocal(rbi[:, b:b + 1], zb[0:1, 0:1])
                rfacp = ps_t.tile([P, P], F32, tag="tp", name=f"rf{b}")
                nc.tensor.matmul(rfacp[0:TPB, 0:1], ones1x16[:, :],
                                 rbi[:, b:b + 1])
                nc.vector.tensor_scalar_mul(
                    outf[:, b, :], e1[:, b, :], rfacp[0:TPB, 0:1]
                )
                nc.sync.dma_start(
                    out=out_ext.ap()[b:b + 1, :].rearrange(
                        "b (t p) -> (b t) p", p=P),
                    in_=outf[:, b, :],
                )

            # ---------------- main loop ----------------
            for t in range(K_TILES):
                b, ti = divmod(t, TPB)
                eps = ps_e.tile([P, DD], F32, tag="ep")
                et = enc_tiles[(b, ti // TPSL)]
                off = (ti % TPSL) * P
                for c in range(NCH):
                    kk = P if c < 4 else K4
                    rr = rhs_main[:, c, :] if c < 4 else rhs4[:, :]
                    for (no, nn) in ((0, 512), (512, 388)):
                        nc.tensor.matmul(
                            eps[:, no:no + nn],
                            et[0:kk, c, off:off + P],
                            rr[:, no:no + nn],
                            start=(c == 0), stop=(c == NCH - 1),
                        )
                z = zp.tile([P, DD], F16, tag="z")
                nc.scalar.activation(z[:, :], eps[:, :], AF.Tanh)
                junk = jp.tile([P, DD], F16, tag="junk")
                nc.vector.scalar_tensor_tensor(
                    out=junk[:, :], in0=z[:, :], scalar=1.0, in1=v_rep[:, :],
                    op0=ALU.mult, op1=ALU.mult,
                    accum_out=scores[:, t:t + 1],
                )

                if K_TILES != NTIL:
                    continue
                # overlapped softmax for the previous batch
                if b >= 1 and ti == 1:
                    emit_exp(b - 1)
                if b >= 1 and ti == 6:
                    emit_tail(b - 1)

            if K_TILES < NTIL:
                return nc

            emit_exp(BL - 1)
            emit_tail(BL - 1)
    return nc


_CACHE = {}


def _get_nc():
    if "nc" not in _CACHE:
        nc = build()
        nc.compile()
        _CACHE["nc"] = nc
    return _CACHE["nc"]


def make_in_maps(hidden, encoder_outputs, attn_W, attn_b, v):
    hidden = np.asarray(hidden, dtype=np.float32)
    encoder_outputs = np.asarray(encoder_outputs, dtype=np.float32)
    attn_W = np.asarray(attn_W, dtype=np.float32)
    attn_b = np.asarray(attn_b, dtype=np.float32)
    v = np.asarray(v, dtype=np.float32)

    WeT = np.ascontiguousarray(attn_W[:, DD:].T)          # [600, 900]
    hb_all = hidden @ attn_W[:, :DD].T + attn_b           # [32, 900]
    v16 = v.astype(np.float16).reshape(1, DD)

    in_maps = []
    for c in range(NCORES):
        bs = slice(c * BL, (c + 1) * BL)
        encp = np.zeros((SROWS, EP), dtype=np.float16)
        encp[:, :E2] = encoder_outputs[bs].reshape(SROWS, E2)
        for b in range(BL):
            encp[b * S:(b + 1) * S, E2 + b] = 1.0
        wcat = np.concatenate([WeT, hb_all[bs]], axis=0).astype(np.float16)
        in_maps.append({
            "enc": encp,
            "wcat": np.ascontiguousarray(wcat),
            "v": v16,
        })
    return in_maps


def run(in_maps, trace=False, **kw):
    nc = _get_nc()
    return run_bass_kernel_spmd(nc, in_maps, core_ids=list(range(NCORES)),
                                trace=trace, **kw)


def kernel(hidden, encoder_outputs, attn_W, attn_b, v):
    in_maps = make_in_maps(hidden, encoder_outputs, attn_W, attn_b, v)
    try:
        res = run(in_maps)
    except Exception:
        # transient device states (e.g. a previously wedged core) sometimes
        # clear on retry
        res = run(in_maps)
    out = np.concatenate([res.results[c]["out"] for c in range(NCORES)], axis=0)
    return np.ascontiguousarray(out, dtype=np.float32)
